# revision 8
# baseline (speedup 1.0000x reference)
"""AggreGCN Trainium2 kernel (8 NeuronCores, SPMD + AllGather).

Math (exact restructuring of the reference):
    x1 = relu(X @ W0 + b0)
    a1 = A_norm @ x1            A_norm = D^-1/2 (Adj + I) D^-1/2
    x2 = relu(a1 @ W1 + b1)     (uses A(XW) = (AX)W linearity)
    a2 = A_norm @ x2
    out = log_softmax(a2 @ W2 + b2)

A_norm @ x is computed as  dinv * segment_sum(y[src] -> dst)  with
y = dinv * x and self-loops included as ordinary edges.

Sharding: nodes are row-sharded over 8 cores (12500 each).  Each layer:
local matmul -> AllGather of y (f32) -> per-core dma_gather of message
rows (512B) from the gathered table, grouped by 32K source-row blocks
(int16 index limit) -> dma_scatter_add (SDMA CCE f32 add) into the
core's local accumulator rows.  All edge plans (gather/scatter index
tiles) are precomputed on the host from edge_index.
"""

import numpy as np
import ml_dtypes

N = 100000
E = 1600000
NFEAT = 256
NHID = 128
NCLASS = 40
NCORES = 8
PER = 12500            # nodes per core
T = 98                 # 128-row tiles per core (PER padded)
PERP = T * 128         # 12544
TS = 7                 # row tiles per super-tile (T % TS == 0)
CHUNK = 4096           # edges per gather/scatter call
BLK = 32768            # gather source block (int16 limit)
NQ = 4                 # SWDGE queues (4 Q7 core pairs in parallel)
SP = False             # single_packet for gather/scatter
YBF16 = True           # y tables (AllGather + gather) in bf16

BF16 = ml_dtypes.bfloat16

_PROG_CACHE = {}


# ----------------------------------------------------------------- host plan

def _wrap_idx(arr, chunk):
    """[n*chunk] int array -> [n, 128, chunk//16] int16 tiles.

    Logical index i of a chunk lives at [i % 16, i // 16]; the 16-row
    block is replicated 8x across the 128 partitions (one copy per
    GPSIMD core)."""
    n = arr.shape[0] // chunk
    a = arr.reshape(n, chunk // 16, 16).astype(np.int16)
    a = np.transpose(a, (0, 2, 1))            # [n, 16, chunk//16]
    return np.tile(a, (1, 8, 1))              # [n, 128, chunk//16]


def _preprocess(edge_index):
    """Build per-core gather/scatter plans and dinv."""
    src = np.concatenate([edge_index[0].astype(np.int64), np.arange(N, dtype=np.int64)])
    dst = np.concatenate([edge_index[1].astype(np.int64), np.arange(N, dtype=np.int64)])
    deg = np.bincount(dst, minlength=N).astype(np.float32)
    dinv = (1.0 / np.sqrt(deg)).astype(np.float32)

    # row of node n in the AllGather output table [NCORES*PERP, NHID]
    loc = (src // PER) * PERP + (src % PER)
    core = dst // PER

    yrows = NCORES * PERP
    nblk = (yrows + BLK - 1) // BLK
    rows_g = [min(BLK, yrows - g * BLK) for g in range(nblk)]

    per_core = []
    counts = np.zeros((NCORES, nblk), dtype=np.int64)
    for c in range(NCORES):
        sel = core == c
        s_loc = loc[sel]
        d_loc = (dst[sel] - c * PER).astype(np.int64)
        blk = s_loc // BLK
        order = np.argsort(blk, kind="stable")
        s_loc, d_loc, blk = s_loc[order], d_loc[order], blk[order]
        for g in range(nblk):
            counts[c, g] = int(np.sum(blk == g))
        per_core.append((s_loc, d_loc, blk))

    nch_g = [max(1, int(-(-counts[:, g].max() // CHUNK))) for g in range(nblk)]
    nch = sum(nch_g)
    dummy_row = T * 128  # row in acc used as trash for padding scatters

    gidx = np.zeros((NCORES, nch, 128, CHUNK // 16), dtype=np.int16)
    sidx = np.zeros((NCORES, nch, 128, CHUNK // 16), dtype=np.int16)
    for c in range(NCORES):
        s_loc, d_loc, blk = per_core[c]
        gs, ss = [], []
        for g in range(nblk):
            m = blk == g
            sg = (s_loc[m] - g * BLK).astype(np.int64)
            dg = d_loc[m]
            padded = nch_g[g] * CHUNK
            pad = padded - sg.shape[0]
            gs.append(np.concatenate([sg, np.zeros(pad, dtype=np.int64)]))
            ss.append(np.concatenate([dg, np.full(pad, dummy_row, dtype=np.int64)]))
        gidx[c] = _wrap_idx(np.concatenate(gs), CHUNK)
        sidx[c] = _wrap_idx(np.concatenate(ss), CHUNK)

    return dinv, gidx, sidx, tuple(nch_g), rows_g


# ------------------------------------------------------------ device program

def _patch_tile_swdge_lanes():
    """Pin each SWDGE queue to its own pair of DMASW sem lanes.

    Tile assigns DMASW completion sems round-robin in scheduler order,
    but the ucode's per-queue shadow-sem tracking requires that one sem
    is only ever incremented by one queue.  With queues used round-robin
    across gather/scatter calls the default assignment mixes queues on a
    lane; pin lane = 2*queue_num + toggle instead (8 lanes / 4 queues)."""
    import concourse.tile_sem_assignment as tsa
    from concourse import bass_isa, mybir
    if getattr(tsa, "_gcn_queue_lanes", False):
        return
    tsa._gcn_queue_lanes = True
    orig = tsa.TileClockTick._assign_tick

    def patched(self, inst):
        if (isinstance(inst, tsa.DMAInst)
                and inst.engine == mybir.EngineType.Pool
                and not isinstance(inst, bass_isa.UserSyncedRemoteDMADescs)):
            q = getattr(inst, "queue_num", 0) or 0
            tog = self.__dict__.setdefault("_gcn_tog", {})
            t = tog.get(q, 0)
            tog[q] = t ^ 1
            self.next_sw_dma_idx = (2 * q + t) % self.swdge_sem_count
        return orig(self, inst)

    tsa.TileClockTick._assign_tick = patched


def _build_program(nch_g, rows_g):
    import concourse.bacc as bacc
    import concourse.mybir as mybir
    import concourse.tile as tile

    _patch_tile_swdge_lanes()

    f32, bf16, i16 = mybir.dt.float32, mybir.dt.bfloat16, mybir.dt.int16
    ADD = mybir.AluOpType.add
    MAX = mybir.AluOpType.max
    SUB = mybir.AluOpType.subtract
    RELU = mybir.ActivationFunctionType.Relu
    EXP = mybir.ActivationFunctionType.Exp
    LN = mybir.ActivationFunctionType.Ln

    nblk = len(nch_g)
    nch = sum(nch_g)
    nsup = T // TS
    accr = (T + 1) * 128

    nc = bacc.Bacc("TRN2", target_bir_lowering=False, debug=False,
                   num_devices=NCORES, num_swdge_queues=NQ)
    ydt = bf16 if YBF16 else f32

    xt = nc.dram_tensor("xt", [NFEAT, PERP], bf16, kind="ExternalInput")
    w0 = nc.dram_tensor("w0", [NFEAT, NHID], bf16, kind="ExternalInput")
    w1 = nc.dram_tensor("w1", [NHID, NHID], bf16, kind="ExternalInput")
    w2 = nc.dram_tensor("w2", [NHID, NCLASS], bf16, kind="ExternalInput")
    b0b = nc.dram_tensor("b0b", [128, NHID], f32, kind="ExternalInput")
    b1b = nc.dram_tensor("b1b", [128, NHID], f32, kind="ExternalInput")
    b2b = nc.dram_tensor("b2b", [128, NCLASS], f32, kind="ExternalInput")
    dinvt = nc.dram_tensor("dinvt", [128, T], f32, kind="ExternalInput")
    identh = nc.dram_tensor("identh", [128, 128], bf16, kind="ExternalInput")
    gidxh = nc.dram_tensor("gidx", [nch, 128, CHUNK // 16], i16, kind="ExternalInput")
    sidxh = nc.dram_tensor("sidx", [nch, 128, CHUNK // 16], i16, kind="ExternalInput")
    outh = nc.dram_tensor("out", [PERP, NCLASS], f32, kind="ExternalOutput")

    y1_in = nc.dram_tensor("y1_in", [PERP, NHID], ydt)
    y2_in = nc.dram_tensor("y2_in", [PERP, NHID], ydt)
    y1_full = nc.dram_tensor("y1_full", [NCORES * PERP, NHID], ydt, addr_space="Shared")
    y2_full = nc.dram_tensor("y2_full", [NCORES * PERP, NHID], ydt, addr_space="Shared")
    acc1 = nc.dram_tensor("acc1", [accr, NHID], f32)
    acc2 = nc.dram_tensor("acc2", [accr, NHID], f32)

    y1_v = y1_in.ap().rearrange("(a p) f -> p a f", p=128)
    y2_v = y2_in.ap().rearrange("(a p) f -> p a f", p=128)
    acc1_v = acc1.ap().rearrange("(a p) f -> p a f", p=128)
    acc2_v = acc2.ap().rearrange("(a p) f -> p a f", p=128)
    out_v = outh.ap().rearrange("(a p) f -> p a f", p=128)

    KT = NFEAT // 128

    with tile.TileContext(nc) as tc:
        with tc.tile_pool(name="const", bufs=1) as cp, \
             tc.tile_pool(name="work", bufs=3) as wp, \
             tc.tile_pool(name="msg", bufs=3) as mp, \
             tc.tile_pool(name="idx", bufs=3) as ip, \
             tc.tile_pool(name="psum", bufs=2, space="PSUM") as pp:

            # ---- constants into SBUF
            w0s = []
            for k in range(KT):
                t_ = cp.tile([128, NHID], bf16, tag=f"w0_{k}")
                nc.sync.dma_start(t_[:], w0[k * 128:(k + 1) * 128, :])
                w0s.append(t_)
            w1s = cp.tile([128, NHID], bf16); nc.sync.dma_start(w1s[:], w1[:, :])
            w2s = cp.tile([128, NCLASS], bf16); nc.sync.dma_start(w2s[:], w2[:, :])
            b0s = cp.tile([128, NHID], f32); nc.sync.dma_start(b0s[:], b0b[:, :])
            b1s = cp.tile([128, NHID], f32); nc.sync.dma_start(b1s[:], b1b[:, :])
            b2s = cp.tile([128, NCLASS], f32); nc.sync.dma_start(b2s[:], b2b[:, :])
            dinvs = cp.tile([128, T], f32); nc.sync.dma_start(dinvs[:], dinvt[:, :])
            ids = cp.tile([128, 128], bf16); nc.sync.dma_start(ids[:], identh[:, :])

            # ---- zero accumulators
            zt = cp.tile([128, TS, NHID], f32)
            nc.vector.memset(zt[:], 0.0)
            for acc_v in (acc1_v, acc2_v):
                for j in range(nsup):
                    nc.sync.dma_start(acc_v[:, j * TS:(j + 1) * TS, :], zt[:])
                nc.sync.dma_start(acc_v[:, T:T + 1, :], zt[:, 0:1, :])

            # ---- stage A: x1 = relu(X@W0+b0); y1 = dinv*x1
            sA = nc.enter_named_scope("l0", False)
            for j in range(nsup):
                xk = []
                for k in range(KT):
                    t_ = wp.tile([128, TS * 128], bf16, tag=f"xk{k}")
                    nc.sync.dma_start(t_[:], xt[k * 128:(k + 1) * 128,
                                                j * TS * 128:(j + 1) * TS * 128])
                    xk.append(t_)
                yst = wp.tile([128, TS, NHID], ydt, tag="yst")
                for u in range(TS):
                    t = j * TS + u
                    ps = pp.tile([128, NHID], f32, tag="mm")
                    for k in range(KT):
                        nc.tensor.matmul(ps[:], lhsT=xk[k][:, u * 128:(u + 1) * 128],
                                         rhs=w0s[k][:], start=(k == 0), stop=(k == KT - 1))
                    x1 = wp.tile([128, NHID], f32, tag="x1")
                    nc.vector.tensor_tensor(x1[:], ps[:], b0s[:], ADD)
                    nc.scalar.activation(x1[:], x1[:], RELU)
                    nc.vector.tensor_scalar_mul(yst[:, u, :], x1[:], dinvs[:, t:t + 1])
                nc.sync.dma_start(y1_v[:, j * TS:(j + 1) * TS, :], yst[:])

            nc.leave_named_scope("l0", sA[0], False)
            # ---- AllGather y1
            sG = nc.enter_named_scope("ag1", False)
            nc.gpsimd.collective_compute(
                "AllGather", mybir.AluOpType.bypass,
                replica_groups=[list(range(NCORES))],
                ins=[y1_in.ap().opt()], outs=[y1_full.ap().opt()])
            nc.leave_named_scope("ag1", sG[0], False)

            # ---- aggregation
            def aggregate(y_full, acc, tag):
                ch = 0
                for g in range(nblk):
                    src_view = y_full[g * BLK: g * BLK + rows_g[g], :]
                    for _ in range(nch_g[g]):
                        q = ch % NQ
                        gi = ip.tile([128, CHUNK // 16], i16, tag="gi")
                        nc.sync.dma_start(gi[:], gidxh[ch])
                        si = ip.tile([128, CHUNK // 16], i16, tag="si")
                        nc.sync.dma_start(si[:], sidxh[ch])
                        msg = mp.tile([128, CHUNK // 128, NHID], ydt, tag="msg")
                        nc.gpsimd.dma_gather(msg[:], src_view, gi[:], CHUNK, CHUNK, NHID,
                                             single_packet=SP, queue_num=q)
                        if YBF16:
                            msgf = mp.tile([128, CHUNK // 128, NHID], f32, tag="msgf")
                            nc.vector.tensor_copy(msgf[:], msg[:])
                        else:
                            msgf = msg
                        nc.gpsimd.dma_scatter_add(acc.ap(), msgf[:], si[:], CHUNK, CHUNK, NHID,
                                                  single_packet=SP, queue_num=q)
                        ch += 1

            s1 = nc.enter_named_scope("agg1", False)
            aggregate(y1_full, acc1, "g1")
            nc.leave_named_scope("agg1", s1[0], False)

            # ---- stage D: x2 = relu(dinv*(acc1@W1)+b1); y2 = dinv*x2
            sD = nc.enter_named_scope("l1", False)
            for j in range(nsup):
                ain = wp.tile([128, TS, NHID], f32, tag="ain")
                nc.sync.dma_start(ain[:], acc1_v[:, j * TS:(j + 1) * TS, :])
                yst = wp.tile([128, TS, NHID], ydt, tag="yst")
                for u in range(TS):
                    t = j * TS + u
                    ab = wp.tile([128, NHID], bf16, tag="ab")
                    nc.vector.tensor_copy(ab[:], ain[:, u, :])
                    pt = pp.tile([128, NHID], bf16, tag="tp")
                    nc.tensor.transpose(pt[:], ab[:], ids[:])
                    aT = wp.tile([128, NHID], bf16, tag="aT")
                    nc.vector.tensor_copy(aT[:], pt[:])
                    ps = pp.tile([128, NHID], f32, tag="mm")
                    nc.tensor.matmul(ps[:], lhsT=aT[:], rhs=w1s[:], start=True, stop=True)
                    x2 = wp.tile([128, NHID], f32, tag="x1")
                    nc.vector.tensor_scalar_mul(x2[:], ps[:], dinvs[:, t:t + 1])
                    nc.vector.tensor_tensor(x2[:], x2[:], b1s[:], ADD)
                    nc.scalar.activation(x2[:], x2[:], RELU)
                    nc.vector.tensor_scalar_mul(yst[:, u, :], x2[:], dinvs[:, t:t + 1])
                nc.sync.dma_start(y2_v[:, j * TS:(j + 1) * TS, :], yst[:])

            nc.leave_named_scope("l1", sD[0], False)
            # ---- AllGather y2 + aggregate
            sG2 = nc.enter_named_scope("ag2", False)
            nc.gpsimd.collective_compute(
                "AllGather", mybir.AluOpType.bypass,
                replica_groups=[list(range(NCORES))],
                ins=[y2_in.ap().opt()], outs=[y2_full.ap().opt()])
            nc.leave_named_scope("ag2", sG2[0], False)
            s2 = nc.enter_named_scope("agg2", False)
            aggregate(y2_full, acc2, "g2")
            nc.leave_named_scope("agg2", s2[0], False)

            # ---- stage H: logits + log_softmax
            sH = nc.enter_named_scope("head", False)
            for j in range(nsup):
                ain = wp.tile([128, TS, NHID], f32, tag="ain")
                nc.sync.dma_start(ain[:], acc2_v[:, j * TS:(j + 1) * TS, :])
                ost = wp.tile([128, TS, NCLASS], f32, tag="ost")
                for u in range(TS):
                    t = j * TS + u
                    ab = wp.tile([128, NHID], bf16, tag="ab")
                    nc.vector.tensor_copy(ab[:], ain[:, u, :])
                    pt = pp.tile([128, NHID], bf16, tag="tp")
                    nc.tensor.transpose(pt[:], ab[:], ids[:])
                    aT = wp.tile([128, NHID], bf16, tag="aT")
                    nc.vector.tensor_copy(aT[:], pt[:])
                    ps = pp.tile([128, NCLASS], f32, tag="mm40")
                    nc.tensor.matmul(ps[:], lhsT=aT[:], rhs=w2s[:], start=True, stop=True)
                    lg = wp.tile([128, NCLASS], f32, tag="lg")
                    nc.vector.tensor_scalar_mul(lg[:], ps[:], dinvs[:, t:t + 1])
                    nc.vector.tensor_tensor(lg[:], lg[:], b2s[:], ADD)
                    nmx = wp.tile([128, 1], f32, tag="nmx")
                    nc.vector.tensor_reduce(nmx[:], lg[:], mybir.AxisListType.X, MAX,
                                            negate=True)
                    ex = wp.tile([128, NCLASS], f32, tag="ex")
                    sm = wp.tile([128, 1], f32, tag="sm")
                    nc.scalar.activation(ex[:], lg[:], EXP, bias=nmx[:], accum_out=sm[:])
                    lsm = wp.tile([128, 1], f32, tag="lsm")
                    nc.scalar.activation(lsm[:], sm[:], LN)
                    tot = wp.tile([128, 1], f32, tag="tot")
                    nc.vector.tensor_tensor(tot[:], lsm[:], nmx[:], SUB)
                    nc.vector.tensor_scalar_sub(ost[:, u, :], lg[:], tot[:])
                nc.sync.dma_start(out_v[:, j * TS:(j + 1) * TS, :], ost[:])
            nc.leave_named_scope("head", sH[0], False)

    nc.compile()
    return nc


# ------------------------------------------------------------------- driver

def _make_in_maps(X, dinv, gidx, sidx, W0, b0, W1, b1, W2, b2):
    ident = np.eye(128, dtype=BF16)
    w0h = np.asarray(W0, dtype=np.float32).astype(BF16)
    w1h = np.asarray(W1, dtype=np.float32).astype(BF16)
    w2h = np.asarray(W2, dtype=np.float32).astype(BF16)
    b0h = np.tile(np.asarray(b0, dtype=np.float32)[None, :], (128, 1))
    b1h = np.tile(np.asarray(b1, dtype=np.float32)[None, :], (128, 1))
    b2h = np.tile(np.asarray(b2, dtype=np.float32)[None, :], (128, 1))

    in_maps = []
    for c in range(NCORES):
        xs = X[c * PER:(c + 1) * PER]
        xtc = np.zeros((NFEAT, PERP), dtype=BF16)
        xtc[:, :PER] = xs.T.astype(BF16)
        dv = np.zeros(PERP, dtype=np.float32)
        dv[:PER] = dinv[c * PER:(c + 1) * PER]
        dvt = dv.reshape(T, 128).T.copy()   # [128, T], [p, t] = dinv[128 t + p]
        in_maps.append({
            "xt": xtc, "w0": w0h, "w1": w1h, "w2": w2h,
            "b0b": b0h, "b1b": b1h, "b2b": b2h,
            "dinvt": dvt, "identh": ident,
            "gidx": gidx[c], "sidx": sidx[c],
        })
    return in_maps


def kernel(aggregated_feature, edge_index, W0, b0, W1, b1, W2, b2):
    from concourse.bass_utils import run_bass_kernel_spmd

    X = np.asarray(aggregated_feature, dtype=np.float32)
    ei = np.asarray(edge_index)
    dinv, gidx, sidx, nch_g, rows_g = _preprocess(ei)

    key = (nch_g, tuple(rows_g), CHUNK, NQ, SP, YBF16)
    if key not in _PROG_CACHE:
        _PROG_CACHE[key] = _build_program(nch_g, rows_g)
    nc = _PROG_CACHE[key]

    in_maps = _make_in_maps(X, dinv, gidx, sidx, W0, b0, W1, b1, W2, b2)
    res = run_bass_kernel_spmd(nc, in_maps, core_ids=list(range(NCORES)))
    out = np.empty((N, NCLASS), dtype=np.float32)
    for c in range(NCORES):
        out[c * PER:(c + 1) * PER] = res.results[c]["out"][:PER]
    return out


# revision 9
# speedup vs baseline: 1.0587x; 1.0587x over previous
"""AggreGCN Trainium2 kernel (8 NeuronCores, SPMD + AllGather).

Math (exact restructuring of the reference):
    x1 = relu(X @ W0 + b0)
    a1 = A_norm @ x1            A_norm = D^-1/2 (Adj + I) D^-1/2
    x2 = relu(a1 @ W1 + b1)     (uses A(XW) = (AX)W linearity)
    a2 = A_norm @ x2
    out = log_softmax(a2 @ W2 + b2)

A_norm @ x is computed as  dinv * segment_sum(y[src] -> dst)  with
y = dinv * x and self-loops included as ordinary edges.

Sharding: nodes are row-sharded over 8 cores (12500 each).  Each layer:
local matmul -> AllGather of y (f32) -> per-core dma_gather of message
rows (512B) from the gathered table, grouped by 32K source-row blocks
(int16 index limit) -> dma_scatter_add (SDMA CCE f32 add) into the
core's local accumulator rows.  All edge plans (gather/scatter index
tiles) are precomputed on the host from edge_index.
"""

import os
import numpy as np
import ml_dtypes

N = 100000
E = 1600000
NFEAT = 256
NHID = 128
NCLASS = 40
NCORES = 8
PER = 12500            # nodes per core
T = 98                 # 128-row tiles per core (PER padded)
PERP = T * 128         # 12544
TS = 7                 # row tiles per super-tile (T % TS == 0)
CHUNK = int(os.environ.get("K_CHUNK", "4096"))   # edges per gather/scatter call
BLK = 32768            # gather source block (int16 limit)
NQ = int(os.environ.get("K_NQ", "4"))            # SWDGE queues
SP = bool(int(os.environ.get("K_SP", "0")))      # single_packet
YBF16 = True           # y tables (AllGather + gather) in bf16

BF16 = ml_dtypes.bfloat16

_PROG_CACHE = {}


# ----------------------------------------------------------------- host plan

def _wrap_idx(arr, chunk):
    """[n*chunk] int array -> [n, 128, chunk//16] int16 tiles.

    Logical index i of a chunk lives at [i % 16, i // 16]; the 16-row
    block is replicated 8x across the 128 partitions (one copy per
    GPSIMD core)."""
    n = arr.shape[0] // chunk
    a = arr.reshape(n, chunk // 16, 16).astype(np.int16)
    a = np.transpose(a, (0, 2, 1))            # [n, 16, chunk//16]
    return np.tile(a, (1, 8, 1))              # [n, 128, chunk//16]


def _preprocess(edge_index):
    """Build per-core gather/scatter plans and dinv."""
    src = np.concatenate([edge_index[0].astype(np.int64), np.arange(N, dtype=np.int64)])
    dst = np.concatenate([edge_index[1].astype(np.int64), np.arange(N, dtype=np.int64)])
    deg = np.bincount(dst, minlength=N).astype(np.float32)
    dinv = (1.0 / np.sqrt(deg)).astype(np.float32)

    # row of node n in the AllGather output table [NCORES*PERP, NHID]
    loc = (src // PER) * PERP + (src % PER)
    core = dst // PER

    yrows = NCORES * PERP
    nblk = (yrows + BLK - 1) // BLK
    rows_g = [min(BLK, yrows - g * BLK) for g in range(nblk)]

    per_core = []
    counts = np.zeros((NCORES, nblk), dtype=np.int64)
    for c in range(NCORES):
        sel = core == c
        s_loc = loc[sel]
        d_loc = (dst[sel] - c * PER).astype(np.int64)
        blk = s_loc // BLK
        order = np.argsort(blk, kind="stable")
        s_loc, d_loc, blk = s_loc[order], d_loc[order], blk[order]
        for g in range(nblk):
            counts[c, g] = int(np.sum(blk == g))
        per_core.append((s_loc, d_loc, blk))

    nch_g = [max(1, int(-(-counts[:, g].max() // CHUNK))) for g in range(nblk)]
    nch = sum(nch_g)
    dummy_row = T * 128  # row in acc used as trash for padding scatters

    gidx = np.zeros((NCORES, nch, 128, CHUNK // 16), dtype=np.int16)
    sidx = np.zeros((NCORES, nch, 128, CHUNK // 16), dtype=np.int16)
    for c in range(NCORES):
        s_loc, d_loc, blk = per_core[c]
        gs, ss = [], []
        for g in range(nblk):
            m = blk == g
            sg = (s_loc[m] - g * BLK).astype(np.int64)
            dg = d_loc[m]
            padded = nch_g[g] * CHUNK
            pad = padded - sg.shape[0]
            gs.append(np.concatenate([sg, np.zeros(pad, dtype=np.int64)]))
            ss.append(np.concatenate([dg, np.full(pad, dummy_row, dtype=np.int64)]))
        gidx[c] = _wrap_idx(np.concatenate(gs), CHUNK)
        sidx[c] = _wrap_idx(np.concatenate(ss), CHUNK)

    return dinv, gidx, sidx, tuple(nch_g), rows_g


# ------------------------------------------------------------ device program

def _patch_tile_swdge_lanes():
    """Pin each SWDGE queue to its own pair of DMASW sem lanes.

    Tile assigns DMASW completion sems round-robin in scheduler order,
    but the ucode's per-queue shadow-sem tracking requires that one sem
    is only ever incremented by one queue.  With queues used round-robin
    across gather/scatter calls the default assignment mixes queues on a
    lane; pin lane = 2*queue_num + toggle instead (8 lanes / 4 queues)."""
    import concourse.tile_sem_assignment as tsa
    from concourse import bass_isa, mybir
    if getattr(tsa, "_gcn_queue_lanes", False):
        return
    tsa._gcn_queue_lanes = True
    orig = tsa.TileClockTick._assign_tick

    def patched(self, inst):
        if (isinstance(inst, tsa.DMAInst)
                and inst.engine == mybir.EngineType.Pool
                and not isinstance(inst, bass_isa.UserSyncedRemoteDMADescs)):
            q = getattr(inst, "queue_num", 0) or 0
            tog = self.__dict__.setdefault("_gcn_tog", {})
            t = tog.get(q, 0)
            tog[q] = t ^ 1
            self.next_sw_dma_idx = (2 * q + t) % self.swdge_sem_count
        return orig(self, inst)

    tsa.TileClockTick._assign_tick = patched


def _build_program(nch_g, rows_g):
    import concourse.bacc as bacc
    import concourse.mybir as mybir
    import concourse.tile as tile

    _patch_tile_swdge_lanes()

    f32, bf16, i16 = mybir.dt.float32, mybir.dt.bfloat16, mybir.dt.int16
    ADD = mybir.AluOpType.add
    MAX = mybir.AluOpType.max
    SUB = mybir.AluOpType.subtract
    RELU = mybir.ActivationFunctionType.Relu
    EXP = mybir.ActivationFunctionType.Exp
    LN = mybir.ActivationFunctionType.Ln

    nblk = len(nch_g)
    nch = sum(nch_g)
    nsup = T // TS
    accr = (T + 1) * 128

    nc = bacc.Bacc("TRN2", target_bir_lowering=False, debug=False,
                   num_devices=NCORES, num_swdge_queues=NQ)
    ydt = bf16 if YBF16 else f32

    xt = nc.dram_tensor("xt", [NFEAT, PERP], bf16, kind="ExternalInput")
    w0 = nc.dram_tensor("w0", [NFEAT, NHID], bf16, kind="ExternalInput")
    w1 = nc.dram_tensor("w1", [NHID, NHID], bf16, kind="ExternalInput")
    w2 = nc.dram_tensor("w2", [NHID, NCLASS], bf16, kind="ExternalInput")
    b0b = nc.dram_tensor("b0b", [128, NHID], f32, kind="ExternalInput")
    b1b = nc.dram_tensor("b1b", [128, NHID], f32, kind="ExternalInput")
    b2b = nc.dram_tensor("b2b", [128, NCLASS], f32, kind="ExternalInput")
    dinvt = nc.dram_tensor("dinvt", [128, T], f32, kind="ExternalInput")
    identh = nc.dram_tensor("identh", [128, 128], bf16, kind="ExternalInput")
    gidxh = nc.dram_tensor("gidx", [nch, 128, CHUNK // 16], i16, kind="ExternalInput")
    sidxh = nc.dram_tensor("sidx", [nch, 128, CHUNK // 16], i16, kind="ExternalInput")
    outh = nc.dram_tensor("out", [PERP, NCLASS], f32, kind="ExternalOutput")

    y1_in = nc.dram_tensor("y1_in", [PERP, NHID], ydt)
    y2_in = nc.dram_tensor("y2_in", [PERP, NHID], ydt)
    y1_full = nc.dram_tensor("y1_full", [NCORES * PERP, NHID], ydt, addr_space="Shared")
    y2_full = nc.dram_tensor("y2_full", [NCORES * PERP, NHID], ydt, addr_space="Shared")
    acc1 = nc.dram_tensor("acc1", [accr, NHID], f32)
    acc2 = nc.dram_tensor("acc2", [accr, NHID], f32)

    y1_v = y1_in.ap().rearrange("(a p) f -> p a f", p=128)
    y2_v = y2_in.ap().rearrange("(a p) f -> p a f", p=128)
    acc1_v = acc1.ap().rearrange("(a p) f -> p a f", p=128)
    acc2_v = acc2.ap().rearrange("(a p) f -> p a f", p=128)
    out_v = outh.ap().rearrange("(a p) f -> p a f", p=128)

    KT = NFEAT // 128

    with tile.TileContext(nc) as tc:
        with tc.tile_pool(name="const", bufs=1) as cp, \
             tc.tile_pool(name="work", bufs=3) as wp, \
             tc.tile_pool(name="msg", bufs=3) as mp, \
             tc.tile_pool(name="idx", bufs=3) as ip, \
             tc.tile_pool(name="psum", bufs=2, space="PSUM") as pp:

            # ---- constants into SBUF
            w0s = []
            for k in range(KT):
                t_ = cp.tile([128, NHID], bf16, tag=f"w0_{k}")
                nc.sync.dma_start(t_[:], w0[k * 128:(k + 1) * 128, :])
                w0s.append(t_)
            w1s = cp.tile([128, NHID], bf16); nc.sync.dma_start(w1s[:], w1[:, :])
            w2s = cp.tile([128, NCLASS], bf16); nc.sync.dma_start(w2s[:], w2[:, :])
            b0s = cp.tile([128, NHID], f32); nc.sync.dma_start(b0s[:], b0b[:, :])
            b1s = cp.tile([128, NHID], f32); nc.sync.dma_start(b1s[:], b1b[:, :])
            b2s = cp.tile([128, NCLASS], f32); nc.sync.dma_start(b2s[:], b2b[:, :])
            dinvs = cp.tile([128, T], f32); nc.sync.dma_start(dinvs[:], dinvt[:, :])
            ids = cp.tile([128, 128], bf16); nc.sync.dma_start(ids[:], identh[:, :])

            # ---- zero accumulators
            zt = cp.tile([128, TS, NHID], f32)
            nc.vector.memset(zt[:], 0.0)
            for acc_v in (acc1_v, acc2_v):
                for j in range(nsup):
                    nc.sync.dma_start(acc_v[:, j * TS:(j + 1) * TS, :], zt[:])
                nc.sync.dma_start(acc_v[:, T:T + 1, :], zt[:, 0:1, :])

            # ---- stage A: x1 = relu(X@W0+b0); y1 = dinv*x1
            sA = nc.enter_named_scope("l0", False)
            for j in range(nsup):
                xk = []
                for k in range(KT):
                    t_ = wp.tile([128, TS * 128], bf16, tag=f"xk{k}")
                    nc.sync.dma_start(t_[:], xt[k * 128:(k + 1) * 128,
                                                j * TS * 128:(j + 1) * TS * 128])
                    xk.append(t_)
                yst = wp.tile([128, TS, NHID], ydt, tag="yst")
                for u in range(TS):
                    t = j * TS + u
                    ps = pp.tile([128, NHID], f32, tag="mm")
                    for k in range(KT):
                        nc.tensor.matmul(ps[:], lhsT=xk[k][:, u * 128:(u + 1) * 128],
                                         rhs=w0s[k][:], start=(k == 0), stop=(k == KT - 1))
                    x1 = wp.tile([128, NHID], f32, tag="x1")
                    nc.vector.tensor_tensor(x1[:], ps[:], b0s[:], ADD)
                    nc.scalar.activation(x1[:], x1[:], RELU)
                    nc.vector.tensor_scalar_mul(yst[:, u, :], x1[:], dinvs[:, t:t + 1])
                nc.sync.dma_start(y1_v[:, j * TS:(j + 1) * TS, :], yst[:])

            nc.leave_named_scope("l0", sA[0], False)
            # ---- AllGather y1
            sG = nc.enter_named_scope("ag1", False)
            nc.gpsimd.collective_compute(
                "AllGather", mybir.AluOpType.bypass,
                replica_groups=[list(range(NCORES))],
                ins=[y1_in.ap().opt()], outs=[y1_full.ap().opt()])
            nc.leave_named_scope("ag1", sG[0], False)

            # ---- aggregation
            def aggregate(y_full, acc, tag):
                ch = 0
                for g in range(nblk):
                    src_view = y_full[g * BLK: g * BLK + rows_g[g], :]
                    for _ in range(nch_g[g]):
                        q = ch % NQ
                        gi = ip.tile([128, CHUNK // 16], i16, tag="gi")
                        nc.sync.dma_start(gi[:], gidxh[ch])
                        si = ip.tile([128, CHUNK // 16], i16, tag="si")
                        nc.sync.dma_start(si[:], sidxh[ch])
                        msg = mp.tile([128, CHUNK // 128, NHID], ydt, tag="msg")
                        nc.gpsimd.dma_gather(msg[:], src_view, gi[:], CHUNK, CHUNK, NHID,
                                             single_packet=SP, queue_num=q)
                        if YBF16:
                            msgf = mp.tile([128, CHUNK // 128, NHID], f32, tag="msgf")
                            nc.vector.tensor_copy(msgf[:], msg[:])
                        else:
                            msgf = msg
                        nc.gpsimd.dma_scatter_add(acc.ap(), msgf[:], si[:], CHUNK, CHUNK, NHID,
                                                  single_packet=SP, queue_num=q)
                        ch += 1

            s1 = nc.enter_named_scope("agg1", False)
            aggregate(y1_full, acc1, "g1")
            nc.leave_named_scope("agg1", s1[0], False)

            # ---- stage D: x2 = relu(dinv*(acc1@W1)+b1); y2 = dinv*x2
            sD = nc.enter_named_scope("l1", False)
            for j in range(nsup):
                ain = wp.tile([128, TS, NHID], f32, tag="ain")
                nc.sync.dma_start(ain[:], acc1_v[:, j * TS:(j + 1) * TS, :])
                yst = wp.tile([128, TS, NHID], ydt, tag="yst")
                for u in range(TS):
                    t = j * TS + u
                    ab = wp.tile([128, NHID], bf16, tag="ab")
                    nc.vector.tensor_copy(ab[:], ain[:, u, :])
                    pt = pp.tile([128, NHID], bf16, tag="tp")
                    nc.tensor.transpose(pt[:], ab[:], ids[:])
                    aT = wp.tile([128, NHID], bf16, tag="aT")
                    nc.vector.tensor_copy(aT[:], pt[:])
                    ps = pp.tile([128, NHID], f32, tag="mm")
                    nc.tensor.matmul(ps[:], lhsT=aT[:], rhs=w1s[:], start=True, stop=True)
                    x2 = wp.tile([128, NHID], f32, tag="x1")
                    nc.vector.tensor_scalar_mul(x2[:], ps[:], dinvs[:, t:t + 1])
                    nc.vector.tensor_tensor(x2[:], x2[:], b1s[:], ADD)
                    nc.scalar.activation(x2[:], x2[:], RELU)
                    nc.vector.tensor_scalar_mul(yst[:, u, :], x2[:], dinvs[:, t:t + 1])
                nc.sync.dma_start(y2_v[:, j * TS:(j + 1) * TS, :], yst[:])

            nc.leave_named_scope("l1", sD[0], False)
            # ---- AllGather y2 + aggregate
            sG2 = nc.enter_named_scope("ag2", False)
            nc.gpsimd.collective_compute(
                "AllGather", mybir.AluOpType.bypass,
                replica_groups=[list(range(NCORES))],
                ins=[y2_in.ap().opt()], outs=[y2_full.ap().opt()])
            nc.leave_named_scope("ag2", sG2[0], False)
            s2 = nc.enter_named_scope("agg2", False)
            aggregate(y2_full, acc2, "g2")
            nc.leave_named_scope("agg2", s2[0], False)

            # ---- stage H: logits + log_softmax
            sH = nc.enter_named_scope("head", False)
            for j in range(nsup):
                ain = wp.tile([128, TS, NHID], f32, tag="ain")
                nc.sync.dma_start(ain[:], acc2_v[:, j * TS:(j + 1) * TS, :])
                ost = wp.tile([128, TS, NCLASS], f32, tag="ost")
                for u in range(TS):
                    t = j * TS + u
                    ab = wp.tile([128, NHID], bf16, tag="ab")
                    nc.vector.tensor_copy(ab[:], ain[:, u, :])
                    pt = pp.tile([128, NHID], bf16, tag="tp")
                    nc.tensor.transpose(pt[:], ab[:], ids[:])
                    aT = wp.tile([128, NHID], bf16, tag="aT")
                    nc.vector.tensor_copy(aT[:], pt[:])
                    ps = pp.tile([128, NCLASS], f32, tag="mm40")
                    nc.tensor.matmul(ps[:], lhsT=aT[:], rhs=w2s[:], start=True, stop=True)
                    lg = wp.tile([128, NCLASS], f32, tag="lg")
                    nc.vector.tensor_scalar_mul(lg[:], ps[:], dinvs[:, t:t + 1])
                    nc.vector.tensor_tensor(lg[:], lg[:], b2s[:], ADD)
                    nmx = wp.tile([128, 1], f32, tag="nmx")
                    nc.vector.tensor_reduce(nmx[:], lg[:], mybir.AxisListType.X, MAX,
                                            negate=True)
                    ex = wp.tile([128, NCLASS], f32, tag="ex")
                    sm = wp.tile([128, 1], f32, tag="sm")
                    nc.scalar.activation(ex[:], lg[:], EXP, bias=nmx[:], accum_out=sm[:])
                    lsm = wp.tile([128, 1], f32, tag="lsm")
                    nc.scalar.activation(lsm[:], sm[:], LN)
                    tot = wp.tile([128, 1], f32, tag="tot")
                    nc.vector.tensor_tensor(tot[:], lsm[:], nmx[:], SUB)
                    nc.vector.tensor_scalar_sub(ost[:, u, :], lg[:], tot[:])
                nc.sync.dma_start(out_v[:, j * TS:(j + 1) * TS, :], ost[:])
            nc.leave_named_scope("head", sH[0], False)

    nc.compile()
    return nc


# ------------------------------------------------------------------- driver

def _make_in_maps(X, dinv, gidx, sidx, W0, b0, W1, b1, W2, b2):
    ident = np.eye(128, dtype=BF16)
    w0h = np.asarray(W0, dtype=np.float32).astype(BF16)
    w1h = np.asarray(W1, dtype=np.float32).astype(BF16)
    w2h = np.asarray(W2, dtype=np.float32).astype(BF16)
    b0h = np.tile(np.asarray(b0, dtype=np.float32)[None, :], (128, 1))
    b1h = np.tile(np.asarray(b1, dtype=np.float32)[None, :], (128, 1))
    b2h = np.tile(np.asarray(b2, dtype=np.float32)[None, :], (128, 1))

    in_maps = []
    for c in range(NCORES):
        xs = X[c * PER:(c + 1) * PER]
        xtc = np.zeros((NFEAT, PERP), dtype=BF16)
        xtc[:, :PER] = xs.T.astype(BF16)
        dv = np.zeros(PERP, dtype=np.float32)
        dv[:PER] = dinv[c * PER:(c + 1) * PER]
        dvt = dv.reshape(T, 128).T.copy()   # [128, T], [p, t] = dinv[128 t + p]
        in_maps.append({
            "xt": xtc, "w0": w0h, "w1": w1h, "w2": w2h,
            "b0b": b0h, "b1b": b1h, "b2b": b2h,
            "dinvt": dvt, "identh": ident,
            "gidx": gidx[c], "sidx": sidx[c],
        })
    return in_maps


def kernel(aggregated_feature, edge_index, W0, b0, W1, b1, W2, b2):
    from concourse.bass_utils import run_bass_kernel_spmd

    X = np.asarray(aggregated_feature, dtype=np.float32)
    ei = np.asarray(edge_index)
    dinv, gidx, sidx, nch_g, rows_g = _preprocess(ei)

    key = (nch_g, tuple(rows_g), CHUNK, NQ, SP, YBF16)
    if key not in _PROG_CACHE:
        _PROG_CACHE[key] = _build_program(nch_g, rows_g)
    nc = _PROG_CACHE[key]

    in_maps = _make_in_maps(X, dinv, gidx, sidx, W0, b0, W1, b1, W2, b2)
    res = run_bass_kernel_spmd(nc, in_maps, core_ids=list(range(NCORES)))
    out = np.empty((N, NCLASS), dtype=np.float32)
    for c in range(NCORES):
        out[c * PER:(c + 1) * PER] = res.results[c]["out"][:PER]
    return out
